# revision 11
# baseline (speedup 1.0000x reference)
"""Lowpass (leaky integrator) scan kernel for Trainium2, 8 NeuronCores.

Recurrence (per feature n, per batch b):
    a_n = exp(-dt / max(tau_n, 1e-8))
    x_t = a_n * x_{t-1} + (1 - a_n) * u_t,   x_{-1} = initial_level_n

Strategy:
  - Data-parallel over batch: 32 batches -> 4 per core, no collectives.
  - On-chip layout: features (N=128) on SBUF partitions, time on the free
    dimension, so the native VectorEngine tensor_tensor_scan instruction
    evaluates the recurrence (state = a*state + v) along time.
  - Input arrives [T, N] (time-major); 128x128 tiles are transposed to
    [N, T] via the TensorEngine (fp32 DMA-transpose unsupported).
  - We scan the rescaled variable z_t = a*z_{t-1} + u_t with
    z_{-1} = x0/(1-a) and x_t = (1-a)*z_t, so the forward transpose feeds
    the scan directly from PSUM with no pre-scaling pass.
  - Back-transpose [N,T] -> [T,N] again on TensorEngine; ScalarEngine
    copies PSUM->SBUF; HWDGE DMA moves 256KB blocks both ways.
"""

import numpy as np
from contextlib import ExitStack

import concourse.bass as bass
import concourse.bacc as bacc
import concourse.mybir as mybir
import concourse.tile as tile
from concourse import masks
from concourse.bass_utils import run_bass_kernel_spmd

DT = 0.001
B, T, N = 32, 4096, 128
NCORES = 8
BC = B // NCORES      # batches per core
TC = 512              # time columns per block (one PSUM bank of fp32)
NSUB = TC // 128      # 128x128 transposes per block
NK = T // TC          # time blocks per batch

_F32 = mybir.dt.float32


def build_nc():
    nc = bacc.Bacc("TRN2", target_bir_lowering=False, debug=False)
    u = nc.declare_dram_parameter("u", [BC, T, N], _F32, isOutput=False)
    tau = nc.declare_dram_parameter("tau", [1, N], _F32, isOutput=False)
    x0 = nc.declare_dram_parameter("x0", [1, N], _F32, isOutput=False)
    y = nc.declare_dram_parameter("y", [BC, T, N], _F32, isOutput=True)

    with tile.TileContext(nc) as tc, ExitStack() as ctx:
        const = ctx.enter_context(tc.tile_pool(name="const", bufs=1))
        in_pool = ctx.enter_context(tc.tile_pool(name="uin", bufs=5))
        z_pool = ctx.enter_context(tc.tile_pool(name="z", bufs=9))
        zs_pool = ctx.enter_context(tc.tile_pool(name="zs", bufs=5))
        out_pool = ctx.enter_context(tc.tile_pool(name="yout", bufs=4))
        pin_pool = ctx.enter_context(tc.tile_pool(name="pin", bufs=4, space="PSUM"))
        pout_pool = ctx.enter_context(tc.tile_pool(name="pout", bufs=2, space="PSUM"))

        ident = const.tile([128, 128], _F32)
        masks.make_identity(nc, ident[:])

        tau_col = const.tile([128, 1], _F32)
        x0_col = const.tile([128, 1], _F32)
        nc.sync.dma_start(tau_col[:], tau[:].rearrange("o n -> n o"))
        nc.sync.dma_start(x0_col[:], x0[:].rearrange("o n -> n o"))

        a_col = const.tile([128, 1], _F32)
        oma_col = const.tile([128, 1], _F32)    # 1 - a
        z0_col = const.tile([128, 1], _F32)     # x0 / (1 - a)
        tmp = const.tile([128, 1], _F32)
        tmp2 = const.tile([128, 1], _F32)

        nc.vector.tensor_scalar_max(tmp[:], tau_col[:], 1e-8)
        nc.vector.reciprocal(tmp[:], tmp[:])
        nc.scalar.activation(
            a_col[:], tmp[:], mybir.ActivationFunctionType.Exp, scale=-DT
        )
        nc.vector.tensor_scalar(
            oma_col[:], a_col[:], -1.0, 1.0,
            op0=mybir.AluOpType.mult, op1=mybir.AluOpType.add,
        )
        nc.vector.reciprocal(tmp2[:], oma_col[:])
        nc.vector.tensor_mul(z0_col[:], x0_col[:], tmp2[:])

        a_bcast = const.tile([128, TC], _F32)
        nc.gpsimd.memset(a_bcast[:], 1.0)
        nc.vector.tensor_scalar_mul(a_bcast[:], a_bcast[:], a_col[:, 0:1])

        # Big blocks of WB = 1024 time steps: two 512-wide scans feed ONE
        # wide ScalarEngine scale, 8 back-transposes, ONE wide PSUM->SBUF
        # copy and ONE 512KB store. The PE back-transposes run PIPE_LAG
        # blocks behind the forward stage so PE never stalls on the
        # scan -> scale chain.
        WB = 2 * TC
        NBLK = T // WB
        PIPE_LAG = 2
        prev = [None] * BC
        pending = []  # (zs_tile, b, kb) awaiting back-transpose + store

        def emit_back(zs, b, kb):
            pout = pout_pool.tile([128, WB], _F32, name="pout")
            for j in range(2 * NSUB):
                nc.tensor.transpose(
                    pout[:, j * 128:(j + 1) * 128],
                    zs[:, j * 128:(j + 1) * 128],
                    ident[:],
                )
            ot = out_pool.tile([128, WB], _F32, name="ot")
            nc.scalar.copy(ot[:], pout[:])
            dst = y[b, kb * WB:(kb + 1) * WB, :].rearrange("(j p) n -> p j n", p=128)
            nc.sync.dma_start(dst, ot[:].rearrange("p (j n) -> p j n", j=2 * NSUB))

        for kb in range(NBLK):
            for b in range(BC):
                ut = in_pool.tile([128, WB], _F32, name="ut")
                src = u[b, kb * WB:(kb + 1) * WB, :].rearrange(
                    "(j p) n -> p j n", p=128
                )
                nc.sync.dma_start(
                    ut[:].rearrange("p (j n) -> p j n", j=2 * NSUB), src
                )

                z = z_pool.tile([128, WB], _F32, name="z")
                for h in range(2):
                    pin = pin_pool.tile([128, TC], _F32, name="pin")
                    for j in range(NSUB):
                        nc.tensor.transpose(
                            pin[:, j * 128:(j + 1) * 128],
                            ut[:, (h * NSUB + j) * 128:(h * NSUB + j + 1) * 128],
                            ident[:],
                        )
                    if h == 0:
                        init = z0_col[:, 0:1] if kb == 0 else prev[b][:, WB - 1:WB]
                    else:
                        init = z[:, TC - 1:TC]
                    nc.vector.tensor_tensor_scan(
                        z[:, h * TC:(h + 1) * TC], a_bcast[:], pin[:], init,
                        mybir.AluOpType.mult, mybir.AluOpType.add,
                    )
                prev[b] = z

                # x = (1-a) * z on the ScalarEngine (per-partition scale)
                zs = zs_pool.tile([128, WB], _F32, name="zs")
                nc.scalar.mul(zs[:], z[:], oma_col[:, 0:1])

                pending.append((zs, b, kb))
                if len(pending) > PIPE_LAG:
                    emit_back(*pending.pop(0))

        for args in pending:
            emit_back(*args)
    nc.compile()
    return nc


_NC = None


def _get_nc():
    global _NC
    if _NC is None:
        _NC = build_nc()
    return _NC


def make_in_maps(inputs, initial_level, tau):
    inputs = np.ascontiguousarray(inputs, dtype=np.float32)
    initial_level = np.ascontiguousarray(initial_level, dtype=np.float32)
    tau = np.ascontiguousarray(tau, dtype=np.float32)
    return [
        {
            "u": inputs[i * BC:(i + 1) * BC],
            "tau": tau,
            "x0": initial_level,
        }
        for i in range(NCORES)
    ]


def kernel(inputs, initial_level, tau):
    nc = _get_nc()
    in_maps = make_in_maps(inputs, initial_level, tau)
    res = run_bass_kernel_spmd(nc, in_maps, list(range(NCORES))).results
    return np.concatenate([res[i]["y"] for i in range(NCORES)], axis=0)


# revision 12
# speedup vs baseline: 1.1045x; 1.1045x over previous
"""Lowpass (leaky integrator) scan kernel for Trainium2, 8 NeuronCores.

Recurrence (per feature n, per batch b):
    a_n = exp(-dt / max(tau_n, 1e-8))
    x_t = a_n * x_{t-1} + (1 - a_n) * u_t,   x_{-1} = initial_level_n

Strategy:
  - Data-parallel over batch: 32 batches -> 4 per core, no collectives.
  - Shard layout: each core's slice is staged feature-major [BC, N, T]
    (transposed at the host shard/unshard boundary), so features (N=128)
    sit on SBUF partitions and time runs along the free dimension. Every
    DMA is then fully contiguous (16KB runs per partition) and the native
    VectorEngine tensor_tensor_scan instruction evaluates the recurrence
    (state = a*state + u) along time directly.
  - We scan the rescaled variable z_t = a*z_{t-1} + u_t with
    z_{-1} = x0/(1-a); the ScalarEngine applies x = (1-a)*z with a
    per-partition scale on the way out.
"""

import numpy as np
from contextlib import ExitStack

import concourse.bacc as bacc
import concourse.mybir as mybir
import concourse.tile as tile
from concourse.bass_utils import run_bass_kernel_spmd

DT = 0.001
B, T, N = 32, 4096, 128
NCORES = 8
BC = B // NCORES      # batches per core
TB = 2048             # time columns per DMA/scale block
SC = 512              # time columns per scan instruction
NH = TB // SC         # scans per block
NBLK = T // TB        # blocks per batch

_F32 = mybir.dt.float32


def build_nc():
    nc = bacc.Bacc("TRN2", target_bir_lowering=False, debug=False)
    u = nc.declare_dram_parameter("u", [BC, N, T], _F32, isOutput=False)
    tau = nc.declare_dram_parameter("tau", [1, N], _F32, isOutput=False)
    x0 = nc.declare_dram_parameter("x0", [1, N], _F32, isOutput=False)
    y = nc.declare_dram_parameter("y", [BC, N, T], _F32, isOutput=True)

    with tile.TileContext(nc) as tc, ExitStack() as ctx:
        const = ctx.enter_context(tc.tile_pool(name="const", bufs=1))
        in_pool = ctx.enter_context(tc.tile_pool(name="uin", bufs=5))
        z_pool = ctx.enter_context(tc.tile_pool(name="z", bufs=9))
        zs_pool = ctx.enter_context(tc.tile_pool(name="zs", bufs=4))

        tau_col = const.tile([128, 1], _F32)
        x0_col = const.tile([128, 1], _F32)
        nc.sync.dma_start(tau_col[:], tau[:].rearrange("o n -> n o"))
        nc.sync.dma_start(x0_col[:], x0[:].rearrange("o n -> n o"))

        a_col = const.tile([128, 1], _F32)
        oma_col = const.tile([128, 1], _F32)    # 1 - a
        z0_col = const.tile([128, 1], _F32)     # x0 / (1 - a)
        tmp = const.tile([128, 1], _F32)
        tmp2 = const.tile([128, 1], _F32)

        nc.vector.tensor_scalar_max(tmp[:], tau_col[:], 1e-8)
        nc.vector.reciprocal(tmp[:], tmp[:])
        nc.scalar.activation(
            a_col[:], tmp[:], mybir.ActivationFunctionType.Exp, scale=-DT
        )
        nc.vector.tensor_scalar(
            oma_col[:], a_col[:], -1.0, 1.0,
            op0=mybir.AluOpType.mult, op1=mybir.AluOpType.add,
        )
        nc.vector.reciprocal(tmp2[:], oma_col[:])
        nc.vector.tensor_mul(z0_col[:], x0_col[:], tmp2[:])

        a_bcast = const.tile([128, SC], _F32)
        nc.gpsimd.memset(a_bcast[:], 1.0)
        nc.vector.tensor_scalar_mul(a_bcast[:], a_bcast[:], a_col[:, 0:1])

        prev = [None] * BC
        for kb in range(NBLK):
            for b in range(BC):
                ut = in_pool.tile([128, TB], _F32, name="ut")
                nc.sync.dma_start(ut[:], u[b, :, kb * TB:(kb + 1) * TB])

                z = z_pool.tile([128, TB], _F32, name="z")
                for h in range(NH):
                    if h == 0:
                        init = z0_col[:, 0:1] if kb == 0 else prev[b][:, TB - 1:TB]
                    else:
                        init = z[:, h * SC - 1:h * SC]
                    nc.vector.tensor_tensor_scan(
                        z[:, h * SC:(h + 1) * SC], a_bcast[:],
                        ut[:, h * SC:(h + 1) * SC], init,
                        mybir.AluOpType.mult, mybir.AluOpType.add,
                    )
                prev[b] = z

                # x = (1-a) * z on the ScalarEngine (per-partition scale)
                zs = zs_pool.tile([128, TB], _F32, name="zs")
                nc.scalar.mul(zs[:], z[:], oma_col[:, 0:1])

                nc.sync.dma_start(y[b, :, kb * TB:(kb + 1) * TB], zs[:])
    nc.compile()
    return nc


_NC = None


def _get_nc():
    global _NC
    if _NC is None:
        _NC = build_nc()
    return _NC


def make_in_maps(inputs, initial_level, tau):
    # Shard layout: feature-major [BC, N, T] per core (contiguous DMA on
    # device); the transpose happens here at the shard boundary.
    inputs_t = np.ascontiguousarray(
        np.asarray(inputs, dtype=np.float32).transpose(0, 2, 1)
    )
    initial_level = np.ascontiguousarray(initial_level, dtype=np.float32)
    tau = np.ascontiguousarray(tau, dtype=np.float32)
    return [
        {
            "u": inputs_t[i * BC:(i + 1) * BC],
            "tau": tau,
            "x0": initial_level,
        }
        for i in range(NCORES)
    ]


def kernel(inputs, initial_level, tau):
    nc = _get_nc()
    in_maps = make_in_maps(inputs, initial_level, tau)
    res = run_bass_kernel_spmd(nc, in_maps, list(range(NCORES))).results
    out_t = np.concatenate([res[i]["y"] for i in range(NCORES)], axis=0)
    return np.ascontiguousarray(out_t.transpose(0, 2, 1))


# revision 13
# speedup vs baseline: 1.3564x; 1.2281x over previous
"""Lowpass (leaky integrator) scan kernel for Trainium2, 8 NeuronCores.

Recurrence (per feature n, per batch b):
    a_n = exp(-dt / max(tau_n, 1e-8))
    x_t = a_n * x_{t-1} + (1 - a_n) * u_t,   x_{-1} = initial_level_n

Strategy:
  - Data-parallel over batch: 32 batches -> 4 per core, no collectives.
  - Shard layout: each core's slice is staged feature-major [BC, N, T]
    (transposed at the host shard/unshard boundary), so features (N=128)
    sit on SBUF partitions and time runs along the free dimension. Every
    DMA is then fully contiguous (16KB runs per partition) and the native
    VectorEngine tensor_tensor_scan instruction evaluates the recurrence
    (state = a*state + u) along time directly.
  - We scan the rescaled variable z_t = a*z_{t-1} + u_t with
    z_{-1} = x0/(1-a); the ScalarEngine applies x = (1-a)*z with a
    per-partition scale on the way out.
"""

import numpy as np
from contextlib import ExitStack

import concourse.bacc as bacc
import concourse.mybir as mybir
import concourse.tile as tile
from concourse.bass_utils import run_bass_kernel_spmd

DT = 0.001
B, T, N = 32, 4096, 128
NCORES = 8
BC = B // NCORES      # batches per core
TB = 1024             # time columns per DMA/scale block
SC = 512              # time columns per scan instruction
NH = TB // SC         # scans per block
NBLK = T // TB        # blocks per batch

_F32 = mybir.dt.float32


def build_nc():
    nc = bacc.Bacc("TRN2", target_bir_lowering=False, debug=False)
    u = nc.declare_dram_parameter("u", [BC, N, T], _F32, isOutput=False)
    tau = nc.declare_dram_parameter("tau", [1, N], _F32, isOutput=False)
    x0 = nc.declare_dram_parameter("x0", [1, N], _F32, isOutput=False)
    y = nc.declare_dram_parameter("y", [BC, N, T], _F32, isOutput=True)

    with tile.TileContext(nc) as tc, ExitStack() as ctx:
        const = ctx.enter_context(tc.tile_pool(name="const", bufs=1))
        in_pool = ctx.enter_context(tc.tile_pool(name="uin", bufs=8))
        z_pool = ctx.enter_context(tc.tile_pool(name="z", bufs=9))
        zs_pool = ctx.enter_context(tc.tile_pool(name="zs", bufs=6))

        tau_col = const.tile([128, 1], _F32)
        x0_col = const.tile([128, 1], _F32)
        nc.sync.dma_start(tau_col[:], tau[:].rearrange("o n -> n o"))
        nc.sync.dma_start(x0_col[:], x0[:].rearrange("o n -> n o"))

        a_col = const.tile([128, 1], _F32)
        oma_col = const.tile([128, 1], _F32)    # 1 - a
        z0_col = const.tile([128, 1], _F32)     # x0 / (1 - a)
        tmp = const.tile([128, 1], _F32)
        tmp2 = const.tile([128, 1], _F32)

        nc.vector.tensor_scalar_max(tmp[:], tau_col[:], 1e-8)
        nc.vector.reciprocal(tmp[:], tmp[:])
        nc.scalar.activation(
            a_col[:], tmp[:], mybir.ActivationFunctionType.Exp, scale=-DT
        )
        nc.vector.tensor_scalar(
            oma_col[:], a_col[:], -1.0, 1.0,
            op0=mybir.AluOpType.mult, op1=mybir.AluOpType.add,
        )
        nc.vector.reciprocal(tmp2[:], oma_col[:])
        nc.vector.tensor_mul(z0_col[:], x0_col[:], tmp2[:])

        a_bcast = const.tile([128, SC], _F32)
        nc.gpsimd.memset(a_bcast[:], 1.0)
        nc.vector.tensor_scalar_mul(a_bcast[:], a_bcast[:], a_col[:, 0:1])

        prev = [None] * BC
        for kb in range(NBLK):
            for b in range(BC):
                ut = in_pool.tile([128, TB], _F32, name="ut")
                nc.sync.dma_start(ut[:], u[b, :, kb * TB:(kb + 1) * TB])

                z = z_pool.tile([128, TB], _F32, name="z")
                for h in range(NH):
                    if h == 0:
                        init = z0_col[:, 0:1] if kb == 0 else prev[b][:, TB - 1:TB]
                    else:
                        init = z[:, h * SC - 1:h * SC]
                    nc.vector.tensor_tensor_scan(
                        z[:, h * SC:(h + 1) * SC], a_bcast[:],
                        ut[:, h * SC:(h + 1) * SC], init,
                        mybir.AluOpType.mult, mybir.AluOpType.add,
                    )
                prev[b] = z

                # x = (1-a) * z on the ScalarEngine (per-partition scale)
                zs = zs_pool.tile([128, TB], _F32, name="zs")
                nc.scalar.mul(zs[:], z[:], oma_col[:, 0:1])

                # separate HWDGE queue (ScalarE) so stores don't FIFO
                # behind prefetched loads on SyncE's queue
                nc.scalar.dma_start(y[b, :, kb * TB:(kb + 1) * TB], zs[:])
    nc.compile()
    return nc


_NC = None


def _get_nc():
    global _NC
    if _NC is None:
        _NC = build_nc()
    return _NC


def make_in_maps(inputs, initial_level, tau):
    # Shard layout: feature-major [BC, N, T] per core (contiguous DMA on
    # device); the transpose happens here at the shard boundary.
    inputs_t = np.ascontiguousarray(
        np.asarray(inputs, dtype=np.float32).transpose(0, 2, 1)
    )
    initial_level = np.ascontiguousarray(initial_level, dtype=np.float32)
    tau = np.ascontiguousarray(tau, dtype=np.float32)
    return [
        {
            "u": inputs_t[i * BC:(i + 1) * BC],
            "tau": tau,
            "x0": initial_level,
        }
        for i in range(NCORES)
    ]


def kernel(inputs, initial_level, tau):
    nc = _get_nc()
    in_maps = make_in_maps(inputs, initial_level, tau)
    res = run_bass_kernel_spmd(nc, in_maps, list(range(NCORES))).results
    out_t = np.concatenate([res[i]["y"] for i in range(NCORES)], axis=0)
    return np.ascontiguousarray(out_t.transpose(0, 2, 1))


# revision 14
# speedup vs baseline: 1.3973x; 1.0301x over previous
"""Lowpass (leaky integrator) scan kernel for Trainium2, 8 NeuronCores.

Recurrence (per feature n, per batch b):
    a_n = exp(-dt / max(tau_n, 1e-8))
    x_t = a_n * x_{t-1} + (1 - a_n) * u_t,   x_{-1} = initial_level_n

Strategy:
  - Data-parallel over batch: 32 batches -> 4 per core, no collectives.
  - Shard layout: each core's slice is staged feature-major [BC, N, T]
    (transposed at the host shard/unshard boundary), so features (N=128)
    sit on SBUF partitions and time runs along the free dimension. Every
    DMA is then fully contiguous (4-16KB runs per partition) and the
    native VectorEngine tensor_tensor_scan instruction evaluates the
    recurrence (state = a*state + u) along time directly at its full
    rate; the scan stream is the critical path.
  - We scan the rescaled variable z_t = a*z_{t-1} + u_t with
    z_{-1} = x0/(1-a); the ScalarEngine applies x = (1-a)*z with a
    per-partition scale on the way out.
  - Loads go out on SyncE's HWDGE queue, stores on ScalarE's, so the two
    streams don't FIFO behind each other.
  - The [1,128] filter coefficients (a, 1-a, x0/(1-a)) are precomputed
    host-side during sharding.
"""

import numpy as np
from contextlib import ExitStack

import concourse.bacc as bacc
import concourse.mybir as mybir
import concourse.tile as tile
from concourse.bass_utils import run_bass_kernel_spmd

DT = 0.001
B, T, N = 32, 4096, 128
NCORES = 8
BC = B // NCORES      # batches per core
TB = 1024             # time columns per DMA/scale block
SC = 512              # time columns per scan instruction
NH = TB // SC         # scans per block
NBLK = T // TB        # blocks per batch

_F32 = mybir.dt.float32


def build_nc():
    nc = bacc.Bacc("TRN2", target_bir_lowering=False, debug=False)
    u = nc.declare_dram_parameter("u", [BC, N, T], _F32, isOutput=False)
    a_in = nc.declare_dram_parameter("a", [1, N], _F32, isOutput=False)
    oma_in = nc.declare_dram_parameter("oma", [1, N], _F32, isOutput=False)
    z0_in = nc.declare_dram_parameter("z0", [1, N], _F32, isOutput=False)
    y = nc.declare_dram_parameter("y", [BC, N, T], _F32, isOutput=True)

    with tile.TileContext(nc) as tc, ExitStack() as ctx:
        const = ctx.enter_context(tc.tile_pool(name="const", bufs=1))
        in_pool = ctx.enter_context(tc.tile_pool(name="uin", bufs=8))
        z_pool = ctx.enter_context(tc.tile_pool(name="z", bufs=9))
        zs_pool = ctx.enter_context(tc.tile_pool(name="zs", bufs=6))

        # First input block goes out on the queue ahead of everything else.
        ut0 = in_pool.tile([128, TB], _F32, name="ut")
        nc.sync.dma_start(ut0[:], u[0, :, 0:TB])

        a_col = const.tile([128, 1], _F32)
        oma_col = const.tile([128, 1], _F32)
        z0_col = const.tile([128, 1], _F32)
        nc.sync.dma_start(a_col[:], a_in[:].rearrange("o n -> n o"))
        nc.sync.dma_start(oma_col[:], oma_in[:].rearrange("o n -> n o"))
        nc.sync.dma_start(z0_col[:], z0_in[:].rearrange("o n -> n o"))

        a_bcast = const.tile([128, SC], _F32)
        nc.gpsimd.memset(a_bcast[:], 1.0)
        nc.vector.tensor_scalar_mul(a_bcast[:], a_bcast[:], a_col[:, 0:1])

        prev = [None] * BC
        for kb in range(NBLK):
            for b in range(BC):
                if kb == 0 and b == 0:
                    ut = ut0
                else:
                    ut = in_pool.tile([128, TB], _F32, name="ut")
                    nc.sync.dma_start(ut[:], u[b, :, kb * TB:(kb + 1) * TB])

                z = z_pool.tile([128, TB], _F32, name="z")
                for h in range(NH):
                    if h == 0:
                        init = z0_col[:, 0:1] if kb == 0 else prev[b][:, TB - 1:TB]
                    else:
                        init = z[:, h * SC - 1:h * SC]
                    nc.vector.tensor_tensor_scan(
                        z[:, h * SC:(h + 1) * SC], a_bcast[:],
                        ut[:, h * SC:(h + 1) * SC], init,
                        mybir.AluOpType.mult, mybir.AluOpType.add,
                    )
                prev[b] = z

                # x = (1-a) * z on the ScalarEngine (per-partition scale);
                # stores ride ScalarE's HWDGE queue, separate from loads.
                last = kb == NBLK - 1 and b == BC - 1
                if not last:
                    zs = zs_pool.tile([128, TB], _F32, name="zs")
                    nc.scalar.mul(zs[:], z[:], oma_col[:, 0:1])
                    nc.scalar.dma_start(y[b, :, kb * TB:(kb + 1) * TB], zs[:])
                else:
                    # split the epilogue of the final block so the first
                    # half's scale/store overlaps the last scan
                    for h in range(NH):
                        zs = zs_pool.tile([128, SC], _F32, name="zsl")
                        nc.scalar.mul(
                            zs[:], z[:, h * SC:(h + 1) * SC], oma_col[:, 0:1]
                        )
                        nc.scalar.dma_start(
                            y[b, :, kb * TB + h * SC:kb * TB + (h + 1) * SC],
                            zs[:],
                        )
    nc.compile()
    return nc


_NC = None


def _get_nc():
    global _NC
    if _NC is None:
        _NC = build_nc()
    return _NC


def make_in_maps(inputs, initial_level, tau):
    # Shard layout: feature-major [BC, N, T] per core (contiguous DMA on
    # device); the transpose happens here at the shard boundary. The tiny
    # [1,128] filter coefficients are precomputed on the host.
    inputs_t = np.ascontiguousarray(
        np.asarray(inputs, dtype=np.float32).transpose(0, 2, 1)
    )
    tau = np.asarray(tau, dtype=np.float32)
    x0 = np.asarray(initial_level, dtype=np.float32)
    a = np.exp(-np.float32(DT) / np.maximum(tau, np.float32(1e-8))).astype(
        np.float32
    )
    oma = (np.float32(1.0) - a).astype(np.float32)
    z0 = (x0 / oma).astype(np.float32)
    return [
        {
            "u": inputs_t[i * BC:(i + 1) * BC],
            "a": a,
            "oma": oma,
            "z0": z0,
        }
        for i in range(NCORES)
    ]


def kernel(inputs, initial_level, tau):
    nc = _get_nc()
    in_maps = make_in_maps(inputs, initial_level, tau)
    res = run_bass_kernel_spmd(nc, in_maps, list(range(NCORES))).results
    out_t = np.concatenate([res[i]["y"] for i in range(NCORES)], axis=0)
    return np.ascontiguousarray(out_t.transpose(0, 2, 1))


# revision 15
# speedup vs baseline: 1.4244x; 1.0194x over previous
"""Lowpass (leaky integrator) scan kernel for Trainium2, 8 NeuronCores.

Recurrence (per feature n, per batch b):
    a_n = exp(-dt / max(tau_n, 1e-8))
    x_t = a_n * x_{t-1} + (1 - a_n) * u_t,   x_{-1} = initial_level_n

Strategy:
  - Data-parallel over batch: 32 batches -> 4 per core, no collectives.
  - Shard layout: each core's slice is staged feature-major [BC, N, T]
    (transposed at the host shard/unshard boundary), so features (N=128)
    sit on SBUF partitions and time runs along the free dimension. Every
    DMA is then fully contiguous (4-16KB runs per partition) and the
    native VectorEngine tensor_tensor_scan instruction evaluates the
    recurrence (state = a*state + u) along time directly at its full
    rate; the scan stream is the critical path.
  - We scan the rescaled variable z_t = a*z_{t-1} + u_t with
    z_{-1} = x0/(1-a); the ScalarEngine applies x = (1-a)*z with a
    per-partition scale on the way out.
  - Loads go out on SyncE's HWDGE queue, stores on ScalarE's, so the two
    streams don't FIFO behind each other.
  - The [1,128] filter coefficients (a, 1-a, x0/(1-a)) are precomputed
    host-side during sharding.
"""

import numpy as np
from contextlib import ExitStack

import concourse.bacc as bacc
import concourse.mybir as mybir
import concourse.tile as tile
from concourse.bass_utils import run_bass_kernel_spmd

DT = 0.001
B, T, N = 32, 4096, 128
NCORES = 8
BC = B // NCORES      # batches per core
TB = 1024             # time columns per DMA/scale block
SC = 512              # time columns per scan instruction
NH = TB // SC         # scans per block
NBLK = T // TB        # blocks per batch

_F32 = mybir.dt.float32


def build_nc():
    nc = bacc.Bacc("TRN2", target_bir_lowering=False, debug=False)
    u = nc.declare_dram_parameter("u", [BC, N, T], _F32, isOutput=False)
    a_in = nc.declare_dram_parameter("a", [1, N], _F32, isOutput=False)
    oma_in = nc.declare_dram_parameter("oma", [1, N], _F32, isOutput=False)
    z0_in = nc.declare_dram_parameter("z0", [1, N], _F32, isOutput=False)
    y = nc.declare_dram_parameter("y", [BC, N, T], _F32, isOutput=True)

    with tile.TileContext(nc) as tc, ExitStack() as ctx:
        const = ctx.enter_context(tc.tile_pool(name="const", bufs=1))
        in_pool = ctx.enter_context(tc.tile_pool(name="uin", bufs=8))
        z_pool = ctx.enter_context(tc.tile_pool(name="z", bufs=9))
        zs_pool = ctx.enter_context(tc.tile_pool(name="zs", bufs=6))

        # First input block goes out on the queue ahead of everything else.
        ut0 = in_pool.tile([128, TB], _F32, name="ut")
        nc.sync.dma_start(ut0[:], u[0, :, 0:TB])

        a_col = const.tile([128, 1], _F32)
        oma_col = const.tile([128, 1], _F32)
        z0_col = const.tile([128, 1], _F32)
        nc.sync.dma_start(a_col[:], a_in[:].rearrange("o n -> n o"))
        nc.sync.dma_start(oma_col[:], oma_in[:].rearrange("o n -> n o"))
        nc.sync.dma_start(z0_col[:], z0_in[:].rearrange("o n -> n o"))

        a_bcast = const.tile([128, SC], _F32)
        nc.gpsimd.memset(a_bcast[:], 1.0)
        nc.vector.tensor_scalar_mul(a_bcast[:], a_bcast[:], a_col[:, 0:1])

        prev = [None] * BC
        for kb in range(NBLK):
            for b in range(BC):
                if kb == 0 and b == 0:
                    ut = ut0
                else:
                    ut = in_pool.tile([128, TB], _F32, name="ut")
                    nc.sync.dma_start(ut[:], u[b, :, kb * TB:(kb + 1) * TB])

                z = z_pool.tile([128, TB], _F32, name="z")
                for h in range(NH):
                    if h == 0:
                        init = z0_col[:, 0:1] if kb == 0 else prev[b][:, TB - 1:TB]
                    else:
                        init = z[:, h * SC - 1:h * SC]
                    nc.vector.tensor_tensor_scan(
                        z[:, h * SC:(h + 1) * SC], a_bcast[:],
                        ut[:, h * SC:(h + 1) * SC], init,
                        mybir.AluOpType.mult, mybir.AluOpType.add,
                    )
                prev[b] = z

                # x = (1-a) * z on the ScalarEngine (per-partition scale);
                # stores ride ScalarE's HWDGE queue, separate from loads.
                last = kb == NBLK - 1 and b == BC - 1
                if not last:
                    zs = zs_pool.tile([128, TB], _F32, name="zs")
                    nc.scalar.mul(zs[:], z[:], oma_col[:, 0:1])
                    nc.scalar.dma_start(y[b, :, kb * TB:(kb + 1) * TB], zs[:])
                else:
                    # split the epilogue of the final block so the first
                    # half's scale/store overlaps the last scan
                    for h in range(NH):
                        zs = zs_pool.tile([128, SC], _F32, name="zsl")
                        nc.scalar.mul(
                            zs[:], z[:, h * SC:(h + 1) * SC], oma_col[:, 0:1]
                        )
                        nc.scalar.dma_start(
                            y[b, :, kb * TB + h * SC:kb * TB + (h + 1) * SC],
                            zs[:],
                        )
    nc.compile()
    return nc


_NC = None


def _get_nc():
    global _NC
    if _NC is None:
        _NC = build_nc()
    return _NC


def make_in_maps(inputs, initial_level, tau):
    # Shard layout: feature-major [BC, N, T] per core (contiguous DMA on
    # device); the transpose happens here at the shard boundary. The tiny
    # [1,128] filter coefficients are precomputed on the host.
    inputs_t = np.ascontiguousarray(
        np.asarray(inputs, dtype=np.float32).transpose(0, 2, 1)
    )
    tau = np.asarray(tau, dtype=np.float32)
    x0 = np.asarray(initial_level, dtype=np.float32)
    # fp32 exp via jax-on-CPU so `a` is bit-identical to the reference's;
    # a 1-ulp difference here is amplified by a^t over long horizons.
    try:
        import jax

        with jax.default_device(jax.local_devices(backend="cpu")[0]):
            a = np.asarray(
                jax.numpy.exp(-DT / jax.numpy.maximum(tau, 1e-8)),
                dtype=np.float32,
            )
    except Exception:
        a = np.exp(-np.float32(DT) / np.maximum(tau, np.float32(1e-8))).astype(
            np.float32
        )
    oma = (np.float32(1.0) - a).astype(np.float32)
    z0 = (x0 / oma).astype(np.float32)
    return [
        {
            "u": inputs_t[i * BC:(i + 1) * BC],
            "a": a,
            "oma": oma,
            "z0": z0,
        }
        for i in range(NCORES)
    ]


def kernel(inputs, initial_level, tau):
    nc = _get_nc()
    in_maps = make_in_maps(inputs, initial_level, tau)
    res = run_bass_kernel_spmd(nc, in_maps, list(range(NCORES))).results
    out_t = np.concatenate([res[i]["y"] for i in range(NCORES)], axis=0)
    return np.ascontiguousarray(out_t.transpose(0, 2, 1))
